# revision 14
# baseline (speedup 1.0000x reference)
"""Trainium2 Bass kernel for HCEN forward: out = ((x.mean(axis=1)) @ W_enc.T + b_enc) @ W_out.T + b_out.

Since there is no nonlinearity between the two linear layers, they fold into
one on host: W_comb = W_out @ W_enc, b_comb = W_out @ b_enc + b_out, so the
device computes out = mean(x) @ W_comb.T + b_comb.

Sharding: data-parallel over batch. B=16 across 8 cores -> 2 batches/core.
x ships as bf16 (16 MB/core); W_comb.T as bf16 in 8 chunk DMAs interleaved
with the early x tiles on the same sync HWDGE ring (a separate-ring weight
DMA gets starved to ~58 GB/s and its completion-sem lane head-of-line blocks
the x stream when the lane is reused).

Per-core pipeline:
  warmup: ~40 tiny PE matmuls during the NEFF preamble so the HAM clock gate
    is at 2.4 GHz when the first tile lands.
  stream x in [128, QT, 1024] bf16 tiles (contiguous 16 KB per partition);
  per q-slab, two ones(=1/S)-stationary matmuls reduce 128 rows into
  psum m[1, 512] chunks. Each (batch, half) accumulation group owns a full
  PSUM bank: interleaved groups sharing one bank corrupt each other
  (observed), separate banks are safe. Trailing tiles are small (QT=2) so
  the post-stream PE tail is short.
  m -> SBUF bf16 per-batch [1, 1024] tiles (partition 0, since ACT/DVE
  cannot write at a partition offset), 8 single-shot PE transposes per batch
  ([1,128] stationary x identity[1,1]) -> mT[128, 8, 2] psum; b0's copies +
  transposes run during b1's stream. One DVE copy -> SBUF, then the combined
  layer mT.T @ W_combT -> out[2, 1024] psum, DVE bias-add, DMA out.
  Host concatenates the 8 [2, 1024] parts.
"""

import os
import sys
from contextlib import ExitStack

import ml_dtypes
import numpy as np

for _p in ("/opt/trn_rl_repo", "/root/.axon_site/_ro/trn_rl_repo"):
    if os.path.isdir(_p) and _p not in sys.path:
        sys.path.insert(0, _p)

import concourse.bass as bass  # noqa: E402
import concourse.tile as tile  # noqa: E402
from concourse import bacc, mybir  # noqa: E402
from concourse.bass_utils import run_bass_kernel_spmd  # noqa: E402


B, S, D, O = 16, 4096, 1024, 1024
NCORES = 8
BPC = B // NCORES  # batches per core
P = 128
DC = D // P
NF = 512  # matmul moving free dim (PSUM bank limit)
F32 = mybir.dt.float32
BF16 = mybir.dt.bfloat16
FP8 = mybir.dt.float8e4

# per-batch s-tiling: q-units of 128 rows each; big tiles first, small last
# so the final tile's PE reduction tail is short.
TILES_B0 = [16, 16]
TILES_B1 = [16, 8, 4, 2, 1, 1]
QBIG = 16
NWARM = 60

_CACHE = {}


def build_nc():
    if "nc" in _CACHE:
        return _CACHE["nc"]
    nc = bacc.Bacc(
        "TRN2",
        target_bir_lowering=False,
        debug=False,
        enable_asserts=False,
        num_devices=NCORES,
    )
    x_ext = nc.dram_tensor("x", [BPC, S, D], FP8, kind="ExternalInput").ap()
    wcombT_ext = nc.dram_tensor("wcombT", [D, O], BF16, kind="ExternalInput").ap()
    bcomb_ext = nc.dram_tensor("bcomb", [O], BF16, kind="ExternalInput").ap()
    out_ext = nc.dram_tensor("out", [BPC, O], F32, kind="ExternalOutput").ap()

    with ExitStack() as ctx:
        tc = ctx.enter_context(tile.TileContext(nc))
        consts = ctx.enter_context(tc.tile_pool(name="consts", bufs=1))
        wpool = ctx.enter_context(tc.tile_pool(name="wpool", bufs=1))
        xbig = ctx.enter_context(tc.tile_pool(name="xbig", bufs=4))
        xsm = ctx.enter_context(tc.tile_pool(name="xsm", bufs=2))
        spool = ctx.enter_context(tc.tile_pool(name="spool", bufs=1))
        pmp = ctx.enter_context(tc.tile_pool(name="pmp", bufs=1, space="PSUM"))
        tpp = ctx.enter_context(tc.tile_pool(name="tpp", bufs=1, space="PSUM"))
        pop = ctx.enter_context(tc.tile_pool(name="pop", bufs=1, space="PSUM"))
        pwp = ctx.enter_context(tc.tile_pool(name="pwp", bufs=1, space="PSUM"))

        ones2 = consts.tile([P, 2, P], FP8)
        nc.vector.memset(ones2[:], 1.0)  # 1/S applied at the psum->SBUF copy
        one1 = consts.tile([1, 1], F32)
        nc.vector.memset(one1[:], 1.0)
        onerow = consts.tile([1, BPC], BF16)
        nc.vector.memset(onerow[:], 1.0)

        # PE warmup: no-dep single-shot matmuls run during the NEFF preamble
        # and first-DMA latency, flipping the HAM clock gate to 2.4 GHz.
        warm_ps = pwp.tile([1, 1], F32, name="warm", tag="warm")
        for _ in range(NWARM):
            nc.tensor.matmul(warm_ps[:], ones2[:, 0, 0:1], ones2[:, 0, 0:1])

        bias_sb = consts.tile([1, O], BF16)

        # phase 1: stream x; per q-slab two ones-stationary matmuls reduce the
        # 128 rows into psum m[1, 512] halves (one PSUM bank per group).
        wcomb_sb = wpool.tile([P, DC, O], BF16)
        pm = [
            [pmp.tile([P, NF], F32, name=f"pm{b}_{n}", tag=f"pm{b}_{n}") for n in range(2)]
            for b in range(BPC)
        ]
        m_sb = [spool.tile([1, D], F32, name=f"m{b}") for b in range(BPC)]
        tp = tpp.tile([P, DC, BPC], F32)
        mt_sb = spool.tile([P, DC, BPC], BF16)
        wchunks = list(range(DC))  # weight chunk DMAs to interleave early

        for b, tiles in ((0, TILES_B0), (1, TILES_B1)):
            nq_total = sum(tiles)
            qdone = 0
            for ti, qt in enumerate(tiles):
                pool = xbig if qt == QBIG else xsm
                xt = pool.tile([P, qt, D], FP8, name=f"xt{qt}", tag=f"xt{qt}")
                s0 = qdone * P
                nc.sync.dma_start(
                    xt[:],
                    x_ext[b, s0 : s0 + P * qt, :].rearrange("(p q) d -> p q d", q=qt),
                )
                if b == 0 and ti == 1:
                    nc.sync.dma_start(bias_sb[:], bcomb_ext[None, :])
                # two weight chunks after each of the first 4 x DMAs
                for _ in range(2):
                    if wchunks:
                        c = wchunks.pop(0)
                        nc.sync.dma_start(
                            wcomb_sb[:, c, :], wcombT_ext[c * P : (c + 1) * P, :]
                        )
                # DoubleRow: each matmul contracts two q-slabs (256 rows);
                # the all-ones stationary is permutation-invariant, so the
                # HW pair-interleave layout cannot scramble the sum.
                for j in range(max(qt // 2, 1)):
                    q0 = 2 * j
                    pair = qt - q0 >= 2
                    for n in range(2):
                        sl = slice(n * NF, (n + 1) * NF)
                        if pair:
                            nc.tensor.matmul(
                                pm[b][n][:],
                                ones2[:],
                                xt[:, q0 : q0 + 2, sl],
                                start=(qdone == 0 and j == 0),
                                stop=(qdone + qt == nq_total and qt - q0 <= 2),
                                perf_mode=mybir.MatmulPerfMode.DoubleRow,
                            )
                        else:
                            nc.tensor.matmul(
                                pm[b][n][:],
                                ones2[:, 0, :],
                                xt[:, q0, sl],
                                start=(qdone == 0 and j == 0),
                                stop=(qdone + qt == nq_total and qt - q0 <= 2),
                            )
                qdone += qt
                for _ in range(2):  # keep the HAM clock gate open between tiles
                    nc.tensor.matmul(warm_ps[:], ones2[:, 0, 0:1], ones2[:, 0, 0:1])

            # as soon as batch b's stream is done: psum m -> SBUF bf16 row
            # (ACT for b0 so it runs during b1's stream, DVE+ACT for b1),
            # then 8 single-shot PE transposes -> tp[:, c, b].
            if b == 0:
                nc.scalar.mul(m_sb[b][0:1, 0:NF], pm[b][0][0:1, :], 1.0 / S)
                nc.scalar.mul(m_sb[b][0:1, NF : 2 * NF], pm[b][1][0:1, :], 1.0 / S)
            else:
                nc.vector.tensor_scalar_mul(m_sb[b][0:1, 0:NF], pm[b][0][0:1, :], 1.0 / S)
                nc.scalar.mul(m_sb[b][0:1, NF : 2 * NF], pm[b][1][0:1, :], 1.0 / S)
            for c in range(DC):
                nc.tensor.transpose(
                    tp[:, c, b : b + 1], m_sb[b][0:1, c * P : (c + 1) * P], one1[:]
                )

        nc.vector.tensor_copy(mt_sb[:], tp[:])

        # combined layer: out[2, 1024] = mT.T @ W_combT (+ bias via DVE)
        out_ps = pop.tile([BPC, O], F32, name="out_ps", tag="ops")
        out_sb = spool.tile([BPC, O], F32)
        for n in range(O // NF):
            sl = slice(n * NF, (n + 1) * NF)
            for c in range(DC):
                nc.tensor.matmul(
                    out_ps[:, sl],
                    mt_sb[:, c, :],
                    wcomb_sb[:, c, sl],
                    start=(c == 0),
                    stop=False,
                )
            # bias folded in as a K=1 rank-1 update: out += ones2x1.T @ bcomb
            nc.tensor.matmul(
                out_ps[:, sl], onerow[:], bias_sb[:, sl], start=False, stop=True
            )
        nc.vector.tensor_copy(out_sb[:, 0:NF], out_ps[:, 0:NF])
        nc.scalar.copy(out_sb[:, NF : 2 * NF], out_ps[:, NF : 2 * NF])
        nc.sync.dma_start(out_ext[:], out_sb[:])

    nc.compile()
    _CACHE["nc"] = nc
    return nc


def make_in_maps(x, W_enc, b_enc, W_out, b_out):
    x = np.asarray(x, dtype=np.float32)
    W_enc = np.asarray(W_enc, dtype=np.float32)
    b_enc = np.asarray(b_enc, dtype=np.float32)
    W_out = np.asarray(W_out, dtype=np.float32)
    b_out = np.asarray(b_out, dtype=np.float32)

    # fold the two linear layers (no nonlinearity between them)
    wcombT = np.ascontiguousarray(
        (W_out @ W_enc).T.astype(ml_dtypes.bfloat16)
    )
    bcomb = np.ascontiguousarray((W_out @ b_enc + b_out).astype(ml_dtypes.bfloat16))
    x16 = x.astype(ml_dtypes.float8_e4m3fn)
    return [
        {
            "x": np.ascontiguousarray(x16[i * BPC : (i + 1) * BPC]),
            "wcombT": wcombT,
            "bcomb": bcomb,
        }
        for i in range(NCORES)
    ]


def gather_out(results):
    return np.ascontiguousarray(
        np.concatenate([results[i]["out"] for i in range(NCORES)], axis=0)
    )


def kernel(x, W_enc, b_enc, W_out, b_out):
    nc = build_nc()
    in_maps = make_in_maps(x, W_enc, b_enc, W_out, b_out)
    res = run_bass_kernel_spmd(nc, in_maps, list(range(NCORES)))
    return gather_out(res.results)


# revision 15
# speedup vs baseline: 1.0062x; 1.0062x over previous
"""Trainium2 Bass kernel for HCEN forward: out = ((x.mean(axis=1)) @ W_enc.T + b_enc) @ W_out.T + b_out.

Since there is no nonlinearity between the two linear layers, they fold into
one on host: W_comb = W_out @ W_enc, b_comb = W_out @ b_enc + b_out, so the
device computes out = mean(x) @ W_comb.T + b_comb.

Sharding: data-parallel over batch. B=16 across 8 cores -> 2 batches/core.
x ships as bf16 (16 MB/core); W_comb.T as bf16 in 8 chunk DMAs interleaved
with the early x tiles on the same sync HWDGE ring (a separate-ring weight
DMA gets starved to ~58 GB/s and its completion-sem lane head-of-line blocks
the x stream when the lane is reused).

Per-core pipeline:
  warmup: ~40 tiny PE matmuls during the NEFF preamble so the HAM clock gate
    is at 2.4 GHz when the first tile lands.
  stream x in [128, QT, 1024] bf16 tiles (contiguous 16 KB per partition);
  per q-slab, two ones(=1/S)-stationary matmuls reduce 128 rows into
  psum m[1, 512] chunks. Each (batch, half) accumulation group owns a full
  PSUM bank: interleaved groups sharing one bank corrupt each other
  (observed), separate banks are safe. Trailing tiles are small (QT=2) so
  the post-stream PE tail is short.
  m -> SBUF bf16 per-batch [1, 1024] tiles (partition 0, since ACT/DVE
  cannot write at a partition offset), 8 single-shot PE transposes per batch
  ([1,128] stationary x identity[1,1]) -> mT[128, 8, 2] psum; b0's copies +
  transposes run during b1's stream. One DVE copy -> SBUF, then the combined
  layer mT.T @ W_combT -> out[2, 1024] psum, DVE bias-add, DMA out.
  Host concatenates the 8 [2, 1024] parts.
"""

import os
import sys
from contextlib import ExitStack

import ml_dtypes
import numpy as np

for _p in ("/opt/trn_rl_repo", "/root/.axon_site/_ro/trn_rl_repo"):
    if os.path.isdir(_p) and _p not in sys.path:
        sys.path.insert(0, _p)

import concourse.bass as bass  # noqa: E402
import concourse.tile as tile  # noqa: E402
from concourse import bacc, mybir  # noqa: E402
from concourse.bass_utils import run_bass_kernel_spmd  # noqa: E402


B, S, D, O = 16, 4096, 1024, 1024
NCORES = 8
BPC = B // NCORES  # batches per core
P = 128
DC = D // P
NF = 512  # matmul moving free dim (PSUM bank limit)
F32 = mybir.dt.float32
BF16 = mybir.dt.bfloat16
FP8 = mybir.dt.float8e4

# per-batch s-tiling: q-units of 128 rows each; big tiles first, small last
# so the final tile's PE reduction tail is short.
TILES_B0 = [8, 8, 8, 8]
TILES_B1 = [8, 8, 8, 4, 2, 1, 1]
QBIG = 8
NWARM = 60

_CACHE = {}


def build_nc():
    if "nc" in _CACHE:
        return _CACHE["nc"]
    nc = bacc.Bacc(
        "TRN2",
        target_bir_lowering=False,
        debug=False,
        enable_asserts=False,
        num_devices=NCORES,
    )
    x_ext = nc.dram_tensor("x", [BPC, S, D], FP8, kind="ExternalInput").ap()
    wcombT_ext = nc.dram_tensor("wcombT", [D, O], BF16, kind="ExternalInput").ap()
    bcomb_ext = nc.dram_tensor("bcomb", [O], BF16, kind="ExternalInput").ap()
    out_ext = nc.dram_tensor("out", [BPC, O], F32, kind="ExternalOutput").ap()

    with ExitStack() as ctx:
        tc = ctx.enter_context(tile.TileContext(nc))
        consts = ctx.enter_context(tc.tile_pool(name="consts", bufs=1))
        wpool = ctx.enter_context(tc.tile_pool(name="wpool", bufs=1))
        xbig = ctx.enter_context(tc.tile_pool(name="xbig", bufs=8))
        xsm = ctx.enter_context(tc.tile_pool(name="xsm", bufs=2))
        spool = ctx.enter_context(tc.tile_pool(name="spool", bufs=1))
        pmp = ctx.enter_context(tc.tile_pool(name="pmp", bufs=1, space="PSUM"))
        tpp = ctx.enter_context(tc.tile_pool(name="tpp", bufs=1, space="PSUM"))
        pop = ctx.enter_context(tc.tile_pool(name="pop", bufs=1, space="PSUM"))
        pwp = ctx.enter_context(tc.tile_pool(name="pwp", bufs=1, space="PSUM"))

        ones2 = consts.tile([P, 2, P], FP8)
        nc.vector.memset(ones2[:], 1.0)  # 1/S applied at the psum->SBUF copy
        one1 = consts.tile([1, 1], F32)
        nc.vector.memset(one1[:], 1.0)
        onerow = consts.tile([1, BPC], BF16)
        nc.vector.memset(onerow[:], 1.0)

        # PE warmup: no-dep single-shot matmuls run during the NEFF preamble
        # and first-DMA latency, flipping the HAM clock gate to 2.4 GHz.
        warm_ps = pwp.tile([1, 1], F32, name="warm", tag="warm")
        for _ in range(NWARM):
            nc.tensor.matmul(warm_ps[:], ones2[:, 0, 0:1], ones2[:, 0, 0:1])

        bias_sb = consts.tile([1, O], BF16)

        # phase 1: stream x; per q-slab two ones-stationary matmuls reduce the
        # 128 rows into psum m[1, 512] halves (one PSUM bank per group).
        wcomb_sb = wpool.tile([P, DC, O], BF16)
        pm = [
            [pmp.tile([P, NF], F32, name=f"pm{b}_{n}", tag=f"pm{b}_{n}") for n in range(2)]
            for b in range(BPC)
        ]
        m_sb = [spool.tile([1, D], F32, name=f"m{b}") for b in range(BPC)]
        tp = tpp.tile([P, DC, BPC], F32)
        mt_sb = spool.tile([P, DC, BPC], BF16)
        wchunks = list(range(DC))  # weight chunk DMAs to interleave early

        for b, tiles in ((0, TILES_B0), (1, TILES_B1)):
            nq_total = sum(tiles)
            qdone = 0
            for ti, qt in enumerate(tiles):
                pool = xbig if qt == QBIG else xsm
                xt = pool.tile([P, qt, D], FP8, name=f"xt{qt}", tag=f"xt{qt}")
                s0 = qdone * P
                nc.sync.dma_start(
                    xt[:],
                    x_ext[b, s0 : s0 + P * qt, :].rearrange("(p q) d -> p q d", q=qt),
                )
                if b == 0 and ti == 1:
                    nc.sync.dma_start(bias_sb[:], bcomb_ext[None, :])
                # two weight chunks after each of the first 4 x DMAs
                for _ in range(2):
                    if wchunks:
                        c = wchunks.pop(0)
                        nc.sync.dma_start(
                            wcomb_sb[:, c, :], wcombT_ext[c * P : (c + 1) * P, :]
                        )
                # DoubleRow: each matmul contracts two q-slabs (256 rows);
                # the all-ones stationary is permutation-invariant, so the
                # HW pair-interleave layout cannot scramble the sum.
                for j in range(max(qt // 2, 1)):
                    q0 = 2 * j
                    pair = qt - q0 >= 2
                    for n in range(2):
                        sl = slice(n * NF, (n + 1) * NF)
                        if pair:
                            nc.tensor.matmul(
                                pm[b][n][:],
                                ones2[:],
                                xt[:, q0 : q0 + 2, sl],
                                start=(qdone == 0 and j == 0),
                                stop=(qdone + qt == nq_total and qt - q0 <= 2),
                                perf_mode=mybir.MatmulPerfMode.DoubleRow,
                            )
                        else:
                            nc.tensor.matmul(
                                pm[b][n][:],
                                ones2[:, 0, :],
                                xt[:, q0, sl],
                                start=(qdone == 0 and j == 0),
                                stop=(qdone + qt == nq_total and qt - q0 <= 2),
                            )
                qdone += qt
                for _ in range(2):  # keep the HAM clock gate open between tiles
                    nc.tensor.matmul(warm_ps[:], ones2[:, 0, 0:1], ones2[:, 0, 0:1])

            # as soon as batch b's stream is done: psum m -> SBUF bf16 row
            # (ACT for b0 so it runs during b1's stream, DVE+ACT for b1),
            # then 8 single-shot PE transposes -> tp[:, c, b].
            if b == 0:
                nc.scalar.mul(m_sb[b][0:1, 0:NF], pm[b][0][0:1, :], 1.0 / S)
                nc.scalar.mul(m_sb[b][0:1, NF : 2 * NF], pm[b][1][0:1, :], 1.0 / S)
            else:
                nc.vector.tensor_scalar_mul(m_sb[b][0:1, 0:NF], pm[b][0][0:1, :], 1.0 / S)
                nc.scalar.mul(m_sb[b][0:1, NF : 2 * NF], pm[b][1][0:1, :], 1.0 / S)
            for c in range(DC):
                nc.tensor.transpose(
                    tp[:, c, b : b + 1], m_sb[b][0:1, c * P : (c + 1) * P], one1[:]
                )

        nc.vector.tensor_copy(mt_sb[:], tp[:])

        # combined layer: out[2, 1024] = mT.T @ W_combT (+ bias via DVE)
        out_ps = pop.tile([BPC, O], F32, name="out_ps", tag="ops")
        out_sb = spool.tile([BPC, O], F32)
        for n in range(O // NF):
            sl = slice(n * NF, (n + 1) * NF)
            for c in range(DC):
                nc.tensor.matmul(
                    out_ps[:, sl],
                    mt_sb[:, c, :],
                    wcomb_sb[:, c, sl],
                    start=(c == 0),
                    stop=False,
                )
            # bias folded in as a K=1 rank-1 update: out += ones2x1.T @ bcomb
            nc.tensor.matmul(
                out_ps[:, sl], onerow[:], bias_sb[:, sl], start=False, stop=True
            )
        nc.vector.tensor_copy(out_sb[:, 0:NF], out_ps[:, 0:NF])
        nc.scalar.copy(out_sb[:, NF : 2 * NF], out_ps[:, NF : 2 * NF])
        nc.sync.dma_start(out_ext[:], out_sb[:])

    nc.compile()
    _CACHE["nc"] = nc
    return nc


def make_in_maps(x, W_enc, b_enc, W_out, b_out):
    x = np.asarray(x, dtype=np.float32)
    W_enc = np.asarray(W_enc, dtype=np.float32)
    b_enc = np.asarray(b_enc, dtype=np.float32)
    W_out = np.asarray(W_out, dtype=np.float32)
    b_out = np.asarray(b_out, dtype=np.float32)

    # fold the two linear layers (no nonlinearity between them)
    wcombT = np.ascontiguousarray(
        (W_out @ W_enc).T.astype(ml_dtypes.bfloat16)
    )
    bcomb = np.ascontiguousarray((W_out @ b_enc + b_out).astype(ml_dtypes.bfloat16))
    x16 = x.astype(ml_dtypes.float8_e4m3fn)
    return [
        {
            "x": np.ascontiguousarray(x16[i * BPC : (i + 1) * BPC]),
            "wcombT": wcombT,
            "bcomb": bcomb,
        }
        for i in range(NCORES)
    ]


def gather_out(results):
    return np.ascontiguousarray(
        np.concatenate([results[i]["out"] for i in range(NCORES)], axis=0)
    )


def kernel(x, W_enc, b_enc, W_out, b_out):
    nc = build_nc()
    in_maps = make_in_maps(x, W_enc, b_enc, W_out, b_out)
    res = run_bass_kernel_spmd(nc, in_maps, list(range(NCORES)))
    return gather_out(res.results)


# revision 16
# speedup vs baseline: 1.1758x; 1.1686x over previous
"""Trainium2 Bass kernel for HCEN forward: out = ((x.mean(axis=1)) @ W_enc.T + b_enc) @ W_out.T + b_out.

Since there is no nonlinearity between the two linear layers, they fold into
one on host: W_comb = W_out @ W_enc, b_comb = W_out @ b_enc + b_out, so the
device computes out = mean(x) @ W_comb.T + b_comb.

Sharding: data-parallel over batch. B=16 across 8 cores -> 2 batches/core.
x ships as bf16 (16 MB/core); W_comb.T as bf16 in 8 chunk DMAs interleaved
with the early x tiles on the same sync HWDGE ring (a separate-ring weight
DMA gets starved to ~58 GB/s and its completion-sem lane head-of-line blocks
the x stream when the lane is reused).

Per-core pipeline:
  warmup: ~40 tiny PE matmuls during the NEFF preamble so the HAM clock gate
    is at 2.4 GHz when the first tile lands.
  stream x in [128, QT, 1024] bf16 tiles (contiguous 16 KB per partition);
  per q-slab, two ones(=1/S)-stationary matmuls reduce 128 rows into
  psum m[1, 512] chunks. Each (batch, half) accumulation group owns a full
  PSUM bank: interleaved groups sharing one bank corrupt each other
  (observed), separate banks are safe. Trailing tiles are small (QT=2) so
  the post-stream PE tail is short.
  m -> SBUF bf16 per-batch [1, 1024] tiles (partition 0, since ACT/DVE
  cannot write at a partition offset), 8 single-shot PE transposes per batch
  ([1,128] stationary x identity[1,1]) -> mT[128, 8, 2] psum; b0's copies +
  transposes run during b1's stream. One DVE copy -> SBUF, then the combined
  layer mT.T @ W_combT -> out[2, 1024] psum, DVE bias-add, DMA out.
  Host concatenates the 8 [2, 1024] parts.
"""

import os
import sys
from contextlib import ExitStack

import ml_dtypes
import numpy as np

for _p in ("/opt/trn_rl_repo", "/root/.axon_site/_ro/trn_rl_repo"):
    if os.path.isdir(_p) and _p not in sys.path:
        sys.path.insert(0, _p)

import concourse.bass as bass  # noqa: E402
import concourse.tile as tile  # noqa: E402
from concourse import bacc, mybir  # noqa: E402
from concourse.bass_utils import run_bass_kernel_spmd  # noqa: E402


B, S, D, O = 16, 4096, 1024, 1024
NCORES = 8
BPC = B // NCORES  # batches per core
P = 128
DC = D // P
NF = 512  # matmul moving free dim (PSUM bank limit)
F32 = mybir.dt.float32
BF16 = mybir.dt.bfloat16
FP8 = mybir.dt.float8e4

# per-batch s-tiling: q-units of 128 rows each; big tiles first, small last
# so the final tile's PE reduction tail is short.
TILES_B0 = [8, 8, 8, 8]
TILES_B1 = [8, 8, 8, 4, 2, 1, 1]
QBIG = 8
NWARM = 19

_CACHE = {}


def build_nc():
    if "nc" in _CACHE:
        return _CACHE["nc"]
    nc = bacc.Bacc(
        "TRN2",
        target_bir_lowering=False,
        debug=False,
        enable_asserts=False,
        num_devices=NCORES,
    )
    x_ext = nc.dram_tensor("x", [BPC, S, D], FP8, kind="ExternalInput").ap()
    wcombT_ext = nc.dram_tensor("wcombT", [D, O], BF16, kind="ExternalInput").ap()
    bcomb_ext = nc.dram_tensor("bcomb", [O], BF16, kind="ExternalInput").ap()
    out_ext = nc.dram_tensor("out", [BPC, O], F32, kind="ExternalOutput").ap()

    with ExitStack() as ctx:
        tc = ctx.enter_context(tile.TileContext(nc))
        consts = ctx.enter_context(tc.tile_pool(name="consts", bufs=1))
        wpool = ctx.enter_context(tc.tile_pool(name="wpool", bufs=1))
        xbig = ctx.enter_context(tc.tile_pool(name="xbig", bufs=8))
        xsm = ctx.enter_context(tc.tile_pool(name="xsm", bufs=2))
        spool = ctx.enter_context(tc.tile_pool(name="spool", bufs=1))
        pmp = ctx.enter_context(tc.tile_pool(name="pmp", bufs=1, space="PSUM"))
        tpp = ctx.enter_context(tc.tile_pool(name="tpp", bufs=1, space="PSUM"))
        pop = ctx.enter_context(tc.tile_pool(name="pop", bufs=1, space="PSUM"))
        pwp = ctx.enter_context(tc.tile_pool(name="pwp", bufs=1, space="PSUM"))

        ones2 = consts.tile([P, 2, P], FP8)
        nc.vector.memset(ones2[:], 1.0)  # 1/S applied at the psum->SBUF copy
        one1 = consts.tile([1, 1], F32)
        nc.vector.memset(one1[:], 1.0)
        onerow = consts.tile([1, BPC], BF16)
        nc.vector.memset(onerow[:], 1.0)

        # PE warmup: the HAM clock gate only unthrottles after one FULLY
        # busy 4096-cycle window, so the warmup must be ~4us of back-to-back
        # full-width matmuls (N=512 DoubleRow on a junk tile), not tiny ones.
        junk = consts.tile([P, 2, NF], FP8)
        nc.vector.memset(junk[:], 1.0)
        warm_ps = pwp.tile([P, NF], F32, name="warm", tag="warm")
        for _ in range(NWARM):
            nc.tensor.matmul(
                warm_ps[:], ones2[:], junk[:],
                perf_mode=mybir.MatmulPerfMode.DoubleRow,
            )

        bias_sb = consts.tile([1, O], BF16)

        # phase 1: stream x; per q-slab two ones-stationary matmuls reduce the
        # 128 rows into psum m[1, 512] halves (one PSUM bank per group).
        wcomb_sb = wpool.tile([P, DC, O], BF16)
        pm = [
            [pmp.tile([P, NF], F32, name=f"pm{b}_{n}", tag=f"pm{b}_{n}") for n in range(2)]
            for b in range(BPC)
        ]
        m_sb = [spool.tile([1, D], F32, name=f"m{b}") for b in range(BPC)]
        tp = tpp.tile([P, DC, BPC], F32)
        mt_sb = spool.tile([P, DC, BPC], BF16)
        wchunks = list(range(DC))  # weight chunk DMAs to interleave early

        for b, tiles in ((0, TILES_B0), (1, TILES_B1)):
            nq_total = sum(tiles)
            qdone = 0
            for ti, qt in enumerate(tiles):
                pool = xbig if qt == QBIG else xsm
                xt = pool.tile([P, qt, D], FP8, name=f"xt{qt}", tag=f"xt{qt}")
                s0 = qdone * P
                nc.sync.dma_start(
                    xt[:],
                    x_ext[b, s0 : s0 + P * qt, :].rearrange("(p q) d -> p q d", q=qt),
                )
                if b == 0 and ti == 1:
                    nc.sync.dma_start(bias_sb[:], bcomb_ext[None, :])
                # two weight chunks after each of the first 4 x DMAs
                for _ in range(2):
                    if wchunks:
                        c = wchunks.pop(0)
                        nc.sync.dma_start(
                            wcomb_sb[:, c, :], wcombT_ext[c * P : (c + 1) * P, :]
                        )
                # DoubleRow: each matmul contracts two q-slabs (256 rows);
                # the all-ones stationary is permutation-invariant, so the
                # HW pair-interleave layout cannot scramble the sum.
                for j in range(max(qt // 2, 1)):
                    q0 = 2 * j
                    pair = qt - q0 >= 2
                    for n in range(2):
                        sl = slice(n * NF, (n + 1) * NF)
                        if pair:
                            nc.tensor.matmul(
                                pm[b][n][:],
                                ones2[:],
                                xt[:, q0 : q0 + 2, sl],
                                start=(qdone == 0 and j == 0),
                                stop=(qdone + qt == nq_total and qt - q0 <= 2),
                                perf_mode=mybir.MatmulPerfMode.DoubleRow,
                            )
                        else:
                            nc.tensor.matmul(
                                pm[b][n][:],
                                ones2[:, 0, :],
                                xt[:, q0, sl],
                                start=(qdone == 0 and j == 0),
                                stop=(qdone + qt == nq_total and qt - q0 <= 2),
                            )
                qdone += qt
                # keep the HAM MID window from seeing a fully idle 3.4us
                nc.tensor.matmul(
                    warm_ps[:], ones2[:], junk[:],
                    perf_mode=mybir.MatmulPerfMode.DoubleRow,
                )

            # as soon as batch b's stream is done: psum m -> SBUF bf16 row
            # (ACT for b0 so it runs during b1's stream, DVE+ACT for b1),
            # then 8 single-shot PE transposes -> tp[:, c, b].
            if b == 0:
                nc.scalar.mul(m_sb[b][0:1, 0:NF], pm[b][0][0:1, :], 1.0 / S)
                nc.scalar.mul(m_sb[b][0:1, NF : 2 * NF], pm[b][1][0:1, :], 1.0 / S)
            else:
                nc.vector.tensor_scalar_mul(m_sb[b][0:1, 0:NF], pm[b][0][0:1, :], 1.0 / S)
                nc.scalar.mul(m_sb[b][0:1, NF : 2 * NF], pm[b][1][0:1, :], 1.0 / S)
            for c in range(DC):
                nc.tensor.transpose(
                    tp[:, c, b : b + 1], m_sb[b][0:1, c * P : (c + 1) * P], one1[:]
                )

        nc.vector.tensor_copy(mt_sb[:], tp[:])

        # combined layer: out[2, 1024] = mT.T @ W_combT (+ bias via DVE)
        out_ps = pop.tile([BPC, O], F32, name="out_ps", tag="ops")
        out_sb = spool.tile([BPC, O], F32)
        for n in range(O // NF):
            sl = slice(n * NF, (n + 1) * NF)
            for c in range(DC):
                nc.tensor.matmul(
                    out_ps[:, sl],
                    mt_sb[:, c, :],
                    wcomb_sb[:, c, sl],
                    start=(c == 0),
                    stop=False,
                )
            # bias folded in as a K=1 rank-1 update: out += ones2x1.T @ bcomb
            nc.tensor.matmul(
                out_ps[:, sl], onerow[:], bias_sb[:, sl], start=False, stop=True
            )
        nc.vector.tensor_copy(out_sb[:, 0:NF], out_ps[:, 0:NF])
        nc.scalar.copy(out_sb[:, NF : 2 * NF], out_ps[:, NF : 2 * NF])
        nc.sync.dma_start(out_ext[:], out_sb[:])

    nc.compile()
    _CACHE["nc"] = nc
    return nc


def make_in_maps(x, W_enc, b_enc, W_out, b_out):
    x = np.asarray(x, dtype=np.float32)
    W_enc = np.asarray(W_enc, dtype=np.float32)
    b_enc = np.asarray(b_enc, dtype=np.float32)
    W_out = np.asarray(W_out, dtype=np.float32)
    b_out = np.asarray(b_out, dtype=np.float32)

    # fold the two linear layers (no nonlinearity between them)
    wcombT = np.ascontiguousarray(
        (W_out @ W_enc).T.astype(ml_dtypes.bfloat16)
    )
    bcomb = np.ascontiguousarray((W_out @ b_enc + b_out).astype(ml_dtypes.bfloat16))
    x16 = x.astype(ml_dtypes.float8_e4m3fn)
    return [
        {
            "x": np.ascontiguousarray(x16[i * BPC : (i + 1) * BPC]),
            "wcombT": wcombT,
            "bcomb": bcomb,
        }
        for i in range(NCORES)
    ]


def gather_out(results):
    return np.ascontiguousarray(
        np.concatenate([results[i]["out"] for i in range(NCORES)], axis=0)
    )


def kernel(x, W_enc, b_enc, W_out, b_out):
    nc = build_nc()
    in_maps = make_in_maps(x, W_enc, b_enc, W_out, b_out)
    res = run_bass_kernel_spmd(nc, in_maps, list(range(NCORES)))
    return gather_out(res.results)
